# revision 13
# baseline (speedup 1.0000x reference)
"""FLAME layer on 8 Trainium2 NeuronCores (Bass/Tile).

Strategy (vertex-parallel):
  * V=5023 padded to 5120 and split 640 vertices/core; every core handles
    the full batch B=2048 for its vertex slice. This replicates only the
    small per-batch operands (betas 3.3MB, pose features, joint transforms)
    while the big model tensor (shapedirs, 24MB) is sliced 1/8 per core.
  * Host (O(B) + O(model) prep): rodrigues, forward-kinematic chain,
    A_rel; J is linear in betas (J = J0 + betas @ SJ with SJ precomputed
    from J_regressor x shapedirs), so no device dependency on v_shaped.
    Host also pre-transposes operands so the device does zero transposes.
  * Device (all O(B*V) work), per 128-vertex chunk in [v(part), b(free)]
    layout:
      1. vposed_c = sdt_aug_c.T @ betas_aug + pdt_c.T @ pose_featT  (PE,
         K=401 incl. a v_template row against a ones row, +K=36 pose)
      2. T = wT.T @ arelT   (PE, K=5) -> 12 planes [v, b] of the blended
         3x4 LBS transform
      3. verts_c = T_c0*vp_x + T_c1*vp_y + T_c2*vp_z + T_c3  (DVE)
      4. DMA out [3, 640, 2048]; host reassembles [B, V, 3].
"""

import os
from contextlib import ExitStack

import numpy as np

import bass_rust
import concourse.bass as bass
import concourse.mybir as mybir
import concourse.tile as tile_mod
from concourse.bass_utils import run_bass_kernel_spmd

# ---------------------------------------------------------------- constants
B = 2048
V = 5023
VP = 5120            # padded vertex count (8 cores x 640)
NVC = VP // 8        # vertices per core
NCHUNK = NVC // 128  # 128-vertex chunks per core (5)
NJ = 5
NCORES = 8
BH = 1024            # batch half (free-dim tile for T/apply)
KCH = [(0, 128), (128, 128), (256, 128), (384, 17)]  # K chunks of 401
PARENTS = np.array([-1, 0, 1, 1, 1])

MM_DT = (mybir.dt.float32r if os.environ.get("FLAME_MM_DT", "f32r") == "f32r"
         else mybir.dt.float32)

# ------------------------------------------------- walrus multi-wait patch
# This walrus build accepts only ONE sem-wait per instruction (CTRL and
# LW queue structs alike), but Tile freely assigns several. Split the
# surplus waits onto same-engine NOPs emitted immediately before the
# instruction — the engine stalls on each NOP's wait first, so the
# gating semantics are identical.


def _patched_commit_instruction(self, inst, lazy_reg_writes=True):
    si = inst.sync_info
    if si is not None and len(si.on_wait) > 1:
        waits = list(si.on_wait)
        inst.sync_info = bass_rust.SyncInfo(
            on_update=list(si.on_update), on_wait=waits[:1]
        )
        for w in waits[1:]:
            nop = mybir.InstNoOp(
                name=self.nc.get_next_instruction_name(),
                engine=inst.engine,
                ins=[],
                outs=[],
                bass_nofuse=True,
                sync_info=bass_rust.SyncInfo(on_update=[], on_wait=[w]),
            )
            _orig_commit_instruction(self, nop, lazy_reg_writes=False)
    return _orig_commit_instruction(self, inst, lazy_reg_writes)


def _split_inst_waits(nc, inst):
    si = inst.ins.sync_info
    if si is None:
        return
    waits = list(si.on_wait)
    if len(waits) <= 1:
        return
    inst.ins.sync_info = bass_rust.SyncInfo(
        on_update=list(si.on_update), on_wait=waits[:1]
    )
    for i in range(1, len(waits)):
        nop = nc.sync.nop(nofuse=True, hint="drain_wait_split")
        nop.ins.sync_info = bass_rust.SyncInfo(on_update=[], on_wait=[waits[i]])


def _patched_drain_and_barrier(self, tick_clock, wait_clock):
    nc = self.nc
    drain_inst = nc.sync.drain()
    wait_clock.add_sem_waits(
        drain_inst.ins, tile_mod.ScopedClock({None: tick_clock.global_clock})
    )
    _split_inst_waits(nc, drain_inst)
    nc.all_engine_barrier()
    assert self.sems is not None
    popped = nc._tile_sem_poison_stack.pop()
    assert popped is self._sem_poison
    nc.clear_and_free_semaphores(list(self.sems.allocated().values()))
    nc.all_engine_barrier()


_orig_commit_instruction = tile_mod.TileContext._commit_instruction
if getattr(tile_mod.TileContext, "_flame_wait_patch", False) is False:
    tile_mod.TileContext._commit_instruction = _patched_commit_instruction
    tile_mod.TileContext._drain_and_barrier = _patched_drain_and_barrier
    tile_mod.TileContext._flame_wait_patch = True

# ----------------------------------------------------------- host-side math


def _rodrigues(r):
    angle = np.linalg.norm(r, axis=-1, keepdims=True) + 1e-8
    axis = r / angle
    x, y, z = axis[..., 0], axis[..., 1], axis[..., 2]
    zero = np.zeros_like(x)
    K = np.stack([zero, -z, y, z, zero, -x, -y, x, zero], axis=-1)
    K = K.reshape(r.shape[:-1] + (3, 3))
    s = np.sin(angle)[..., None]
    c = np.cos(angle)[..., None]
    return np.eye(3, dtype=r.dtype) + s * K + (1.0 - c) * (K @ K)


def _host_batch_prep(shape, expression, rotation, neck, jaw, eyeballs,
                     v_template, shapedirs, J_regressor):
    f64 = np.float64
    b = shape.shape[0]
    betas = np.concatenate([shape, expression], axis=1).astype(f64)
    full_pose = np.concatenate([rotation, neck, jaw, eyeballs], axis=1).astype(f64)

    jr = J_regressor.astype(f64)
    J0 = jr @ v_template.astype(f64)                                   # [5,3]
    SJ = np.einsum('jv,vcl->ljc', jr, shapedirs.astype(f64)).reshape(400, NJ * 3)
    J = (J0.reshape(-1) + betas @ SJ).reshape(b, NJ, 3)

    rot_mats = _rodrigues(full_pose.reshape(b, NJ, 3))
    pose_feature = (rot_mats[:, 1:] - np.eye(3, dtype=f64)).reshape(b, 36)

    rel_joints = np.concatenate([J[:, :1], J[:, 1:] - J[:, PARENTS[1:]]], axis=1)
    T_local = np.zeros((b, NJ, 4, 4), dtype=f64)
    T_local[:, :, :3, :3] = rot_mats
    T_local[:, :, :3, 3] = rel_joints
    T_local[:, :, 3, 3] = 1.0
    chain = [T_local[:, 0]]
    for j in range(1, NJ):
        chain.append(chain[PARENTS[j]] @ T_local[:, j])
    A = np.stack(chain, axis=1)

    j_hom = np.concatenate([J, np.zeros_like(J[..., :1])], axis=-1)
    t_corr = np.einsum('bjmn,bjn->bjm', A, j_hom)
    A_rel = A.copy()
    A_rel[:, :, :, 3] -= t_corr

    # [60, B]: rows (j, c, n) — K=60 contraction layout (8KB/partition
    # in SBUF instead of 96KB for a [5, 12*B] layout)
    arelT = np.ascontiguousarray(
        A_rel[:, :, :3, :].transpose(1, 2, 3, 0).reshape(NJ * 12, b)
    ).astype(np.float32)
    betasT_aug = np.concatenate(
        [betas.T, np.ones((1, b), dtype=f64)], axis=0).astype(np.float32)
    pfT = np.ascontiguousarray(pose_feature.T).astype(np.float32)
    return arelT, betasT_aug, pfT


def _host_model_prep(v_template, shapedirs, posedirs, lbs_weights):
    sdt = np.zeros((3, 401, VP), dtype=np.float32)
    sdt[:, :400, :V] = shapedirs.transpose(1, 2, 0)
    sdt[:, 400, :V] = v_template.T
    pdt = np.zeros((3, 36, VP), dtype=np.float32)
    pdt[:, :, :V] = posedirs.reshape(36, V, 3).transpose(2, 0, 1)
    # w60: zero-structured lhsT for the K=60 T-blend, laid out
    # [60 rows (j,c,n), 12 planes (c,n), VP]: for output plane (c,n),
    # row j*12 + c*4 + n carries lbs_weights[v, j]
    w60 = np.zeros((NJ * 12, 12, VP), dtype=np.float32)
    for cn in range(12):
        for j in range(NJ):
            w60[j * 12 + cn, cn, :V] = lbs_weights[:, j]
    return sdt, pdt, w60

# ------------------------------------------------------------ device kernel


def _build_device_program():
    nc = bass.Bass("TRN2", target_bir_lowering=False, debug=False)
    dt = mybir.dt.float32

    sdt = nc.dram_tensor("sdt", [3, 401, NVC], dt, kind="ExternalInput").ap()
    pdt = nc.dram_tensor("pdt", [3, 36, NVC], dt, kind="ExternalInput").ap()
    wt = nc.dram_tensor("wt", [NJ * 12, 12 * NVC], dt, kind="ExternalInput").ap()
    betas = nc.dram_tensor("betas", [401, B], dt, kind="ExternalInput").ap()
    pf = nc.dram_tensor("pf", [36, B], dt, kind="ExternalInput").ap()
    arel = nc.dram_tensor("arel", [NJ * 12, B], dt, kind="ExternalInput").ap()
    out = nc.dram_tensor("out", [3, NVC, B], dt, kind="ExternalOutput").ap()

    with tile_mod.TileContext(nc) as tc, ExitStack() as ctx:
        cpool = ctx.enter_context(tc.tile_pool(name="const", bufs=1))
        spool = ctx.enter_context(tc.tile_pool(name="stream", bufs=2))
        vpool = ctx.enter_context(tc.tile_pool(name="vposed", bufs=2))
        tpool = ctx.enter_context(tc.tile_pool(name="tblend", bufs=1))
        apool = ctx.enter_context(tc.tile_pool(name="apply", bufs=2))
        ps_v = ctx.enter_context(tc.tile_pool(name="psv", bufs=4, space="PSUM"))
        ps_t = ctx.enter_context(tc.tile_pool(name="pst", bufs=4, space="PSUM"))

        # resident operands
        bt = []
        for ki, (k0, kn) in enumerate(KCH):
            t = cpool.tile([kn, B], dt, tag=f"bt{ki}")
            nc.sync.dma_start(t[:], betas[k0:k0 + kn, :])
            bt.append(t)
        pft = cpool.tile([36, B], dt, tag="pft")
        nc.sync.dma_start(pft[:], pf[:, :])
        arelt = cpool.tile([NJ * 12, B], dt, tag="arelt")
        nc.sync.dma_start(arelt[:], arel[:, :])
        wtt = cpool.tile([NJ * 12, 12 * NVC], dt, tag="wtt")
        nc.sync.dma_start(wtt[:], wt[:, :])

        for k in range(NCHUNK):
            vs = slice(k * 128, (k + 1) * 128)

            # stream this chunk's lhsT tiles
            sdt_t = []
            for c in range(3):
                row = []
                for ki, (k0, kn) in enumerate(KCH):
                    t = spool.tile([kn, 128], dt, tag=f"sdt{c}_{ki}")
                    nc.sync.dma_start(t[:], sdt[c, k0:k0 + kn, vs])
                    row.append(t)
                sdt_t.append(row)
            pdt_t = []
            for c in range(3):
                t = spool.tile([36, 128], dt, tag=f"pdt{c}")
                nc.sync.dma_start(t[:], pdt[c, :, vs])
                pdt_t.append(t)

            # 1) vposed planes [128, B]
            vp = []
            for c in range(3):
                dst = vpool.tile([128, B], dt, tag=f"vp{c}")
                for ns in range(B // 512):
                    bs = slice(ns * 512, (ns + 1) * 512)
                    acc = ps_v.tile([128, 512], dt, tag="psv")
                    for ki in range(4):
                        nc.tensor.matmul(
                            acc[:], lhsT=sdt_t[c][ki][:].bitcast(MM_DT),
                            rhs=bt[ki][:, bs].bitcast(MM_DT),
                            start=(ki == 0), stop=False)
                    nc.tensor.matmul(
                        acc[:], lhsT=pdt_t[c][:].bitcast(MM_DT),
                        rhs=pft[:, bs].bitcast(MM_DT),
                        start=False, stop=True)
                    nc.scalar.copy(out=dst[:, bs], in_=acc[:])
                vp.append(dst)

            # 2+3) per batch-half: T blend then affine apply
            for h in range(B // BH):
                hb = slice(h * BH, (h + 1) * BH)
                tt = tpool.tile([128, 12 * BH], dt, tag="tt")
                for cn in range(12):
                    wsl = wtt[:, cn * NVC + k * 128:cn * NVC + (k + 1) * 128]
                    for ns in range(BH // 512):
                        fs = slice(cn * BH + ns * 512, cn * BH + (ns + 1) * 512)
                        src = slice(h * BH + ns * 512, h * BH + (ns + 1) * 512)
                        acc = ps_t.tile([128, 512], dt, tag="pst")
                        nc.tensor.matmul(
                            acc[:], lhsT=wsl.bitcast(MM_DT),
                            rhs=arelt[:, src].bitcast(MM_DT),
                            start=True, stop=True)
                        nc.scalar.copy(out=tt[:, fs], in_=acc[:])

                for c in range(3):
                    def tsl(n):
                        return tt[:, (4 * c + n) * BH:(4 * c + n + 1) * BH]
                    ma = apool.tile([128, BH], dt, tag="ma")
                    mb = apool.tile([128, BH], dt, tag="mb")
                    nc.vector.tensor_mul(ma[:], tsl(0), vp[0][:, hb])
                    nc.vector.tensor_mul(mb[:], tsl(1), vp[1][:, hb])
                    nc.vector.tensor_add(ma[:], ma[:], mb[:])
                    nc.vector.tensor_mul(mb[:], tsl(2), vp[2][:, hb])
                    nc.vector.tensor_add(mb[:], mb[:], tsl(3))
                    nc.vector.tensor_add(ma[:], ma[:], mb[:])
                    nc.sync.dma_start(out[c, vs, hb], ma[:])
    return nc


_NC_CACHE = {}


def _get_nc():
    if "nc" not in _NC_CACHE:
        _NC_CACHE["nc"] = _build_device_program()
    return _NC_CACHE["nc"]

# ---------------------------------------------------------------- entry


def build_in_maps(shape, expression, rotation, neck, jaw, eyeballs,
                  v_template, shapedirs, posedirs, J_regressor, lbs_weights):
    arelT, betasT_aug, pfT = _host_batch_prep(
        shape, expression, rotation, neck, jaw, eyeballs,
        v_template, shapedirs, J_regressor)
    sdt, pdt, w60 = _host_model_prep(v_template, shapedirs, posedirs, lbs_weights)

    in_maps = []
    for i in range(NCORES):
        v0, v1 = i * NVC, (i + 1) * NVC
        in_maps.append({
            "sdt": np.ascontiguousarray(sdt[:, :, v0:v1]),
            "pdt": np.ascontiguousarray(pdt[:, :, v0:v1]),
            "wt": np.ascontiguousarray(w60[:, :, v0:v1]).reshape(NJ * 12, 12 * NVC),
            "betas": betasT_aug,
            "pf": pfT,
            "arel": arelT,
        })
    return in_maps


def kernel(shape, expression, rotation, neck, jaw, eyeballs,
           v_template, shapedirs, posedirs, J_regressor, lbs_weights):
    in_maps = build_in_maps(shape, expression, rotation, neck, jaw, eyeballs,
                            v_template, shapedirs, posedirs, J_regressor,
                            lbs_weights)
    nc = _get_nc()
    res = run_bass_kernel_spmd(nc, in_maps, core_ids=list(range(NCORES)))

    full = np.concatenate([res.results[i]["out"] for i in range(NCORES)], axis=1)
    verts = np.ascontiguousarray(full[:, :V, :].transpose(2, 1, 0))
    return verts.astype(np.float32)


# revision 14
# speedup vs baseline: 1.7514x; 1.7514x over previous
"""FLAME layer on 8 Trainium2 NeuronCores (Bass/Tile).

Strategy (vertex-parallel):
  * V=5023 padded to 5120 and split 640 vertices/core; every core handles
    the full batch B=2048 for its vertex slice. This replicates only the
    small per-batch operands (betas 3.3MB, pose features, joint transforms)
    while the big model tensor (shapedirs, 24MB) is sliced 1/8 per core.
  * Host (O(B) + O(model) prep): rodrigues, forward-kinematic chain,
    A_rel; J is linear in betas (J = J0 + betas @ SJ with SJ precomputed
    from J_regressor x shapedirs), so no device dependency on v_shaped.
    Host also pre-transposes operands so the device does zero transposes.
  * Device (all O(B*V) work), per 128-vertex chunk in [v(part), b(free)]
    layout:
      1. vposed_c = sdt_aug_c.T @ betas_aug + pdt_c.T @ pose_featT  (PE,
         K=401 incl. a v_template row against a ones row, +K=36 pose)
      2. T = wT.T @ arelT   (PE, K=5) -> 12 planes [v, b] of the blended
         3x4 LBS transform
      3. verts_c = T_c0*vp_x + T_c1*vp_y + T_c2*vp_z + T_c3  (DVE)
      4. DMA out [3, 640, 2048]; host reassembles [B, V, 3].
"""

import os
from contextlib import ExitStack

import numpy as np

import bass_rust
import concourse.bass as bass
import concourse.mybir as mybir
import concourse.tile as tile_mod
from concourse.bass_utils import run_bass_kernel_spmd

# ---------------------------------------------------------------- constants
B = 2048
V = 5023
VP = 5120            # padded vertex count (8 cores x 640)
NVC = VP // 8        # vertices per core
NCHUNK = NVC // 128  # 128-vertex chunks per core (5)
NJ = 5
NCORES = 8
BH = 1024            # batch half (free-dim tile for T/apply)
KCH = [(0, 128), (128, 128), (256, 128), (384, 17)]  # K chunks of 401
PARENTS = np.array([-1, 0, 1, 1, 1])

MM_DT = (mybir.dt.float32r if os.environ.get("FLAME_MM_DT", "f32r") == "f32r"
         else mybir.dt.float32)

# ------------------------------------------------- walrus multi-wait patch
# This walrus build accepts only ONE sem-wait per instruction (CTRL and
# LW queue structs alike), but Tile freely assigns several. Split the
# surplus waits onto same-engine NOPs emitted immediately before the
# instruction — the engine stalls on each NOP's wait first, so the
# gating semantics are identical.


def _patched_commit_instruction(self, inst, lazy_reg_writes=True):
    si = inst.sync_info
    if si is not None and len(si.on_wait) > 1:
        waits = list(si.on_wait)
        inst.sync_info = bass_rust.SyncInfo(
            on_update=list(si.on_update), on_wait=waits[:1]
        )
        for w in waits[1:]:
            nop = mybir.InstNoOp(
                name=self.nc.get_next_instruction_name(),
                engine=inst.engine,
                ins=[],
                outs=[],
                bass_nofuse=True,
                sync_info=bass_rust.SyncInfo(on_update=[], on_wait=[w]),
            )
            _orig_commit_instruction(self, nop, lazy_reg_writes=False)
    return _orig_commit_instruction(self, inst, lazy_reg_writes)


def _split_inst_waits(nc, inst):
    si = inst.ins.sync_info
    if si is None:
        return
    waits = list(si.on_wait)
    if len(waits) <= 1:
        return
    inst.ins.sync_info = bass_rust.SyncInfo(
        on_update=list(si.on_update), on_wait=waits[:1]
    )
    for i in range(1, len(waits)):
        nop = nc.sync.nop(nofuse=True, hint="drain_wait_split")
        nop.ins.sync_info = bass_rust.SyncInfo(on_update=[], on_wait=[waits[i]])


def _patched_drain_and_barrier(self, tick_clock, wait_clock):
    nc = self.nc
    drain_inst = nc.sync.drain()
    wait_clock.add_sem_waits(
        drain_inst.ins, tile_mod.ScopedClock({None: tick_clock.global_clock})
    )
    _split_inst_waits(nc, drain_inst)
    nc.all_engine_barrier()
    assert self.sems is not None
    popped = nc._tile_sem_poison_stack.pop()
    assert popped is self._sem_poison
    nc.clear_and_free_semaphores(list(self.sems.allocated().values()))
    nc.all_engine_barrier()


_orig_commit_instruction = tile_mod.TileContext._commit_instruction
if getattr(tile_mod.TileContext, "_flame_wait_patch", False) is False:
    tile_mod.TileContext._commit_instruction = _patched_commit_instruction
    tile_mod.TileContext._drain_and_barrier = _patched_drain_and_barrier
    tile_mod.TileContext._flame_wait_patch = True

# ----------------------------------------------------------- host-side math


def _rodrigues(r):
    angle = np.linalg.norm(r, axis=-1, keepdims=True) + 1e-8
    axis = r / angle
    x, y, z = axis[..., 0], axis[..., 1], axis[..., 2]
    zero = np.zeros_like(x)
    K = np.stack([zero, -z, y, z, zero, -x, -y, x, zero], axis=-1)
    K = K.reshape(r.shape[:-1] + (3, 3))
    s = np.sin(angle)[..., None]
    c = np.cos(angle)[..., None]
    return np.eye(3, dtype=r.dtype) + s * K + (1.0 - c) * (K @ K)


def _host_batch_prep(shape, expression, rotation, neck, jaw, eyeballs,
                     v_template, shapedirs, J_regressor):
    f64 = np.float64
    b = shape.shape[0]
    betas = np.concatenate([shape, expression], axis=1).astype(f64)
    full_pose = np.concatenate([rotation, neck, jaw, eyeballs], axis=1).astype(f64)

    jr = J_regressor.astype(f64)
    J0 = jr @ v_template.astype(f64)                                   # [5,3]
    SJ = np.einsum('jv,vcl->ljc', jr, shapedirs.astype(f64)).reshape(400, NJ * 3)
    J = (J0.reshape(-1) + betas @ SJ).reshape(b, NJ, 3)

    rot_mats = _rodrigues(full_pose.reshape(b, NJ, 3))
    pose_feature = (rot_mats[:, 1:] - np.eye(3, dtype=f64)).reshape(b, 36)

    rel_joints = np.concatenate([J[:, :1], J[:, 1:] - J[:, PARENTS[1:]]], axis=1)
    T_local = np.zeros((b, NJ, 4, 4), dtype=f64)
    T_local[:, :, :3, :3] = rot_mats
    T_local[:, :, :3, 3] = rel_joints
    T_local[:, :, 3, 3] = 1.0
    chain = [T_local[:, 0]]
    for j in range(1, NJ):
        chain.append(chain[PARENTS[j]] @ T_local[:, j])
    A = np.stack(chain, axis=1)

    j_hom = np.concatenate([J, np.zeros_like(J[..., :1])], axis=-1)
    t_corr = np.einsum('bjmn,bjn->bjm', A, j_hom)
    A_rel = A.copy()
    A_rel[:, :, :, 3] -= t_corr

    # [60, B]: rows (j, c, n) — K=60 contraction layout (8KB/partition
    # in SBUF instead of 96KB for a [5, 12*B] layout)
    arelT = np.ascontiguousarray(
        A_rel[:, :, :3, :].transpose(1, 2, 3, 0).reshape(NJ * 12, b)
    ).astype(np.float32)
    betasT_aug = np.concatenate(
        [betas.T, np.ones((1, b), dtype=f64)], axis=0).astype(np.float32)
    pfT = np.ascontiguousarray(pose_feature.T).astype(np.float32)
    return arelT, betasT_aug, pfT


def _host_model_prep(v_template, shapedirs, posedirs, lbs_weights):
    sdt = np.zeros((3, 401, VP), dtype=np.float32)
    sdt[:, :400, :V] = shapedirs.transpose(1, 2, 0)
    sdt[:, 400, :V] = v_template.T
    pdt = np.zeros((3, 36, VP), dtype=np.float32)
    pdt[:, :, :V] = posedirs.reshape(36, V, 3).transpose(2, 0, 1)
    # w60: zero-structured lhsT for the K=60 T-blend, laid out
    # [60 rows (j,c,n), 12 planes (c,n), VP]: for output plane (c,n),
    # row j*12 + c*4 + n carries lbs_weights[v, j]
    w60 = np.zeros((NJ * 12, 12, VP), dtype=np.float32)
    for cn in range(12):
        for j in range(NJ):
            w60[j * 12 + cn, cn, :V] = lbs_weights[:, j]
    return sdt, pdt, w60

# ------------------------------------------------------------ device kernel


def _build_device_program():
    nc = bass.Bass("TRN2", target_bir_lowering=False, debug=False)
    dt = mybir.dt.float32

    mdt = MM_DT
    sdt = nc.dram_tensor("sdt", [3, 401, NVC], mdt, kind="ExternalInput").ap()
    pdt = nc.dram_tensor("pdt", [3, 36, NVC], mdt, kind="ExternalInput").ap()
    wt = nc.dram_tensor("wt", [NJ * 12, 12 * NVC], mdt, kind="ExternalInput").ap()
    betas = nc.dram_tensor("betas", [401, B], mdt, kind="ExternalInput").ap()
    pf = nc.dram_tensor("pf", [36, B], mdt, kind="ExternalInput").ap()
    arel = nc.dram_tensor("arel", [NJ * 12, B], mdt, kind="ExternalInput").ap()
    out = nc.dram_tensor("out", [3, NVC, B], dt, kind="ExternalOutput").ap()

    with tile_mod.TileContext(nc) as tc, ExitStack() as ctx:
        cpool = ctx.enter_context(tc.tile_pool(name="const", bufs=1))
        spool = ctx.enter_context(tc.tile_pool(name="stream", bufs=2))
        vpool = ctx.enter_context(tc.tile_pool(name="vposed", bufs=2))
        tpool = ctx.enter_context(tc.tile_pool(name="tblend", bufs=1))
        apool = ctx.enter_context(tc.tile_pool(name="apply", bufs=2))
        ps_v = ctx.enter_context(tc.tile_pool(name="psv", bufs=4, space="PSUM"))
        ps_t = ctx.enter_context(tc.tile_pool(name="pst", bufs=4, space="PSUM"))

        # resident operands
        bt = []
        for ki, (k0, kn) in enumerate(KCH):
            t = cpool.tile([kn, B], mdt, tag=f"bt{ki}")
            nc.sync.dma_start(t[:], betas[k0:k0 + kn, :])
            bt.append(t)
        pft = cpool.tile([36, B], mdt, tag="pft")
        nc.sync.dma_start(pft[:], pf[:, :])
        arelt = cpool.tile([NJ * 12, B], mdt, tag="arelt")
        nc.sync.dma_start(arelt[:], arel[:, :])
        wtt = cpool.tile([NJ * 12, 12 * NVC], mdt, tag="wtt")
        nc.sync.dma_start(wtt[:], wt[:, :])

        for k in range(NCHUNK):
            vs = slice(k * 128, (k + 1) * 128)

            # stream this chunk's lhsT tiles
            sdt_t = []
            for c in range(3):
                row = []
                for ki, (k0, kn) in enumerate(KCH):
                    t = spool.tile([kn, 128], mdt, tag=f"sdt{c}_{ki}")
                    nc.sync.dma_start(t[:], sdt[c, k0:k0 + kn, vs])
                    row.append(t)
                sdt_t.append(row)
            pdt_t = []
            for c in range(3):
                t = spool.tile([36, 128], mdt, tag=f"pdt{c}")
                nc.sync.dma_start(t[:], pdt[c, :, vs])
                pdt_t.append(t)

            # 1) vposed planes [128, B]
            vp = []
            for c in range(3):
                dst = vpool.tile([128, B], dt, tag=f"vp{c}")
                for ns in range(B // 512):
                    bs = slice(ns * 512, (ns + 1) * 512)
                    acc = ps_v.tile([128, 512], dt, tag="psv")
                    for ki in range(4):
                        nc.tensor.matmul(
                            acc[:], lhsT=sdt_t[c][ki][:],
                            rhs=bt[ki][:, bs],
                            start=(ki == 0), stop=False)
                    nc.tensor.matmul(
                        acc[:], lhsT=pdt_t[c][:],
                        rhs=pft[:, bs],
                        start=False, stop=True)
                    nc.scalar.copy(out=dst[:, bs], in_=acc[:])
                vp.append(dst)

            # 2+3) per batch-half: T blend then affine apply
            for h in range(B // BH):
                hb = slice(h * BH, (h + 1) * BH)
                tt = tpool.tile([128, 12 * BH], dt, tag="tt")
                for cn in range(12):
                    wsl = wtt[:, cn * NVC + k * 128:cn * NVC + (k + 1) * 128]
                    for ns in range(BH // 512):
                        fs = slice(cn * BH + ns * 512, cn * BH + (ns + 1) * 512)
                        src = slice(h * BH + ns * 512, h * BH + (ns + 1) * 512)
                        acc = ps_t.tile([128, 512], dt, tag="pst")
                        nc.tensor.matmul(
                            acc[:], lhsT=wsl,
                            rhs=arelt[:, src],
                            start=True, stop=True)
                        nc.scalar.copy(out=tt[:, fs], in_=acc[:])

                for c in range(3):
                    def tsl(n):
                        return tt[:, (4 * c + n) * BH:(4 * c + n + 1) * BH]
                    ma = apool.tile([128, BH], dt, tag="ma")
                    mb = apool.tile([128, BH], dt, tag="mb")
                    nc.vector.tensor_mul(ma[:], tsl(0), vp[0][:, hb])
                    nc.vector.tensor_mul(mb[:], tsl(1), vp[1][:, hb])
                    nc.vector.tensor_add(ma[:], ma[:], mb[:])
                    nc.vector.tensor_mul(mb[:], tsl(2), vp[2][:, hb])
                    nc.vector.tensor_add(mb[:], mb[:], tsl(3))
                    nc.vector.tensor_add(ma[:], ma[:], mb[:])
                    nc.sync.dma_start(out[c, vs, hb], ma[:])
    return nc


_NC_CACHE = {}


def _get_nc():
    if "nc" not in _NC_CACHE:
        _NC_CACHE["nc"] = _build_device_program()
    return _NC_CACHE["nc"]

# ---------------------------------------------------------------- entry


def build_in_maps(shape, expression, rotation, neck, jaw, eyeballs,
                  v_template, shapedirs, posedirs, J_regressor, lbs_weights):
    arelT, betasT_aug, pfT = _host_batch_prep(
        shape, expression, rotation, neck, jaw, eyeballs,
        v_template, shapedirs, J_regressor)
    sdt, pdt, w60 = _host_model_prep(v_template, shapedirs, posedirs, lbs_weights)

    in_maps = []
    for i in range(NCORES):
        v0, v1 = i * NVC, (i + 1) * NVC
        in_maps.append({
            "sdt": np.ascontiguousarray(sdt[:, :, v0:v1]),
            "pdt": np.ascontiguousarray(pdt[:, :, v0:v1]),
            "wt": np.ascontiguousarray(w60[:, :, v0:v1]).reshape(NJ * 12, 12 * NVC),
            "betas": betasT_aug,
            "pf": pfT,
            "arel": arelT,
        })
    return in_maps


def kernel(shape, expression, rotation, neck, jaw, eyeballs,
           v_template, shapedirs, posedirs, J_regressor, lbs_weights):
    in_maps = build_in_maps(shape, expression, rotation, neck, jaw, eyeballs,
                            v_template, shapedirs, posedirs, J_regressor,
                            lbs_weights)
    nc = _get_nc()
    res = run_bass_kernel_spmd(nc, in_maps, core_ids=list(range(NCORES)))

    full = np.concatenate([res.results[i]["out"] for i in range(NCORES)], axis=1)
    verts = np.ascontiguousarray(full[:, :V, :].transpose(2, 1, 0))
    return verts.astype(np.float32)
